# revision 18
# baseline (speedup 1.0000x reference)
"""HGT (heterogeneous graph transformer) on 8 Trainium2 NeuronCores.

Single-NEFF, single-launch design; everything runs on device.

  - Dst sharding: core c owns users [c*2500,(c+1)*2500) padded to 2560 and
    items [c*6250,(c+1)*6250) padded to 6656.  All cores run the SAME
    program; per-core behavior lives entirely in the data (index arrays,
    input shards), never in the instruction stream.
  - Per layer every core computes the FULL folded k/v/q tables: the kqv
    projection with the relation transforms A_k (scaled by p_rel/sqrt(D))
    and A_v folded in as block-diagonal factors, so per-edge work reduces
    to gathers.
  - Edge phase (local-dst edges, sorted by dst, per-bin padded so the
    slice plan is identical on all cores): dma_gather of fused [k|v]
    source rows and q dst rows, per-head dot + exp on chip, then a
    one-hot PE-matmul segment-sum aggregates the weighted values AND the
    softmax denominators per dst row into PSUM bins.  Pad tokens carry
    dstv=-1 which zeroes their one-hot column.  (dma_scatter_add is NOT
    used: its CCE read-modify-write loses colliding updates.)
  - Softmax division, exact gelu, output projection and the gated skip run
    per 128-row bin straight out of PSUM; layer-0 results are exchanged
    with AllGather collectives, layer-1 results flow directly into the
    final shared linear.
  - No max subtraction in the segment softmax: score ranges on this model
    are |s| < 5 (measured), so plain exp() is exact to f32 roundoff.

Tables with >32767 rows are gathered with a pair trick: rows viewed as
[n/2, 2*w], idx = padded_row >> 1 fits int16, and the correct half is
selected on-chip from the row parity (broadcast multiply of the hi-lo
difference by a per-token parity mask).

Tables are fp16 (DVE 2x mode + half the gather bytes) except q_u, whose
128-element rows must stay f32 to keep gather descriptors at 512B.
"""

import sys
import numpy as np

sys.path.insert(0, "/opt/trn_rl_repo")

H, D = 8, 16
HID = 128
NU, NI = 20000, 50000
L = 2
NC = 8
P = 128

LU, LUP = 2500, 2560        # local users, padded -> 20 bins
LI, LIP = 6250, 6656        # local items, padded -> 52 bins
NUP = NC * LUP              # 20480
NIP = NC * LIP              # 53248
NBU = LUP // P
NBI = LIP // P
CH = 8                      # slices per chunk -> 1024 tokens
TC = CH * P

_prog_cache = {}
_HW_NS_TOTAL = 0
_LAST_RES = None


# ---------------------------------------------------------------------------
# host-side helpers
# ---------------------------------------------------------------------------

def _blockdiag(blocks):
    out = np.zeros((HID, HID), dtype=np.float32)
    for h in range(H):
        out[h * D:(h + 1) * D, h * D:(h + 1) * D] = blocks[h]
    return out


def _wrap16(idx):
    a = np.asarray(idx, np.int16).reshape(-1, 16).T
    return np.ascontiguousarray(a)


def _wrap128(vals, dtype=np.float32):
    a = np.asarray(vals, dtype).reshape(-1, P).T
    return np.ascontiguousarray(a)


def _pad_users(u):
    u = np.asarray(u, np.int64)
    return (u // LU) * LUP + u % LU


def _pad_items(i):
    i = np.asarray(i, np.int64)
    return (i // LI) * LIP + i % LI


class Stream:
    """One relation stream for one sink: dst-sorted, per-bin padded."""

    def __init__(self, name, n_bins):
        self.name = name
        self.n_bins = n_bins

    def plan(self, per_core_counts):
        cnt = np.asarray(per_core_counts)          # [NC, n_bins]
        self.S = np.maximum(np.ceil(cnt.max(axis=0) / P).astype(int), 1)
        self.bin_slices = []
        g = 0
        for b in range(self.n_bins):
            self.bin_slices.append(list(range(g, g + int(self.S[b]))))
            g += int(self.S[b])
        self.n_slices = -(-g // CH) * CH           # pad to chunk multiple
        self.n_tok = self.n_slices * P
        self.n_chunks = self.n_slices // CH

    def fill(self, src_kv, kv_par, src_q, q_par, dst_loc):
        """Token arrays for one core (append). All args are np int/float."""
        order = np.argsort(dst_loc, kind="stable")
        src_kv, src_q, dst_loc = src_kv[order], src_q[order], dst_loc[order]
        kv_par = kv_par[order] if kv_par is not None else None
        q_par = q_par[order] if q_par is not None else None
        bins = dst_loc // P
        skv = np.zeros(self.n_tok, np.int64)
        sq = np.zeros(self.n_tok, np.int64)
        dv = np.full(self.n_tok, -1.0, np.float32)
        pkv = np.zeros(self.n_tok, np.float32)
        pq = np.zeros(self.n_tok, np.float32)
        pos = np.searchsorted(bins, np.arange(self.n_bins + 1))
        for b in range(self.n_bins):
            lo, hi = int(pos[b]), int(pos[b + 1])
            n = hi - lo
            t0 = self.bin_slices[b][0] * P
            assert n <= int(self.S[b]) * P, (self.name, b, n, self.S[b])
            skv[t0:t0 + n] = src_kv[lo:hi]
            sq[t0:t0 + n] = src_q[lo:hi]
            dv[t0:t0 + n] = (dst_loc[lo:hi] % P).astype(np.float32)
            if kv_par is not None:
                pkv[t0:t0 + n] = kv_par[lo:hi]
            if q_par is not None:
                pq[t0:t0 + n] = q_par[lo:hi]
        if not hasattr(self, "src16"):
            self.src16, self.qdst16, self.dstv = [], [], []
            self.kvpar, self.qpar = [], []
        self.src16.append(_wrap16(skv))
        self.qdst16.append(_wrap16(sq))
        self.dstv.append(_wrap128(dv))
        self.kvpar.append(_wrap128(pkv))
        self.qpar.append(_wrap128(pq))


# ---------------------------------------------------------------------------
# device program
# ---------------------------------------------------------------------------

def _build_program(streams, g_gate):
    """g_gate: dict ('u'|'i', layer) -> float sigmoid(skip)."""
    import concourse.bacc as bacc
    import concourse.mybir as mybir
    import concourse.tile as tile

    ui, iu, uu = streams["ui"], streams["iu"], streams["uu"]
    dt = mybir.dt
    AF = mybir.ActivationFunctionType
    OP = mybir.AluOpType

    nc = bacc.Bacc("TRN2", target_bir_lowering=False, debug=False,
                   num_devices=NC)

    # ---- I/O --------------------------------------------------------------
    xu0 = nc.dram_tensor("xu0", [P, LUP], dt.float32, kind="ExternalInput")
    xi0 = nc.dram_tensor("xi0", [64, LIP], dt.float32, kind="ExternalInput")
    w_in_u = nc.dram_tensor("w_in_u", [P, P], dt.float32, kind="ExternalInput")
    w_in_i = nc.dram_tensor("w_in_i", [64, P], dt.float32, kind="ExternalInput")
    b_in_u = nc.dram_tensor("b_in_u", [P, 1], dt.float32, kind="ExternalInput")
    b_in_i = nc.dram_tensor("b_in_i", [P, 1], dt.float32, kind="ExternalInput")
    wbig_u = nc.dram_tensor("wbig_u", [L, P, 640], dt.float32, kind="ExternalInput")
    bbig_u = nc.dram_tensor("bbig_u", [L, 1, 640], dt.float32, kind="ExternalInput")
    wbig_i = nc.dram_tensor("wbig_i", [L, P, 384], dt.float32, kind="ExternalInput")
    bbig_i = nc.dram_tensor("bbig_i", [L, 1, 384], dt.float32, kind="ExternalInput")
    w_out_u = nc.dram_tensor("w_out_u", [L, P, P], dt.float32, kind="ExternalInput")
    w_out_i = nc.dram_tensor("w_out_i", [L, P, P], dt.float32, kind="ExternalInput")
    b_out_u = nc.dram_tensor("b_out_u", [L, P, 1], dt.float32, kind="ExternalInput")
    b_out_i = nc.dram_tensor("b_out_i", [L, P, 1], dt.float32, kind="ExternalInput")
    w_lin = nc.dram_tensor("w_lin", [P, P], dt.float32, kind="ExternalInput")
    b_lin = nc.dram_tensor("b_lin", [1, P], dt.float32, kind="ExternalInput")
    ident = nc.dram_tensor("ident", [P, P], dt.float32, kind="ExternalInput")

    idx_in = {}
    for st in (ui, iu, uu):
        idx_in[st.name] = dict(
            src=nc.dram_tensor(f"{st.name}_src", [16, st.n_tok // 16], dt.int16,
                               kind="ExternalInput"),
            qdst=nc.dram_tensor(f"{st.name}_qdst", [16, st.n_tok // 16], dt.int16,
                                kind="ExternalInput"),
            dstv=nc.dram_tensor(f"{st.name}_dstv", [P, st.n_tok // P], dt.float32,
                                kind="ExternalInput"),
        )
    idx_in["iu"]["kvpar"] = nc.dram_tensor(
        "iu_kvpar", [P, iu.n_tok // P], dt.float32, kind="ExternalInput")
    idx_in["ui"]["qpar"] = nc.dram_tensor(
        "ui_qpar", [P, ui.n_tok // P], dt.float32, kind="ExternalInput")

    out = nc.dram_tensor("out", [LUP + LIP, 64], dt.float32, kind="ExternalOutput")

    with tile.TileContext(nc) as tc:
        with (
            tc.tile_pool(name="const", bufs=1) as cp,
            tc.tile_pool(name="dram", bufs=1, space="DRAM") as dram,
            tc.tile_pool(name="hload", bufs=2) as hp,
            tc.tile_pool(name="s1o", bufs=2) as s1p,
            tc.tile_pool(name="edge", bufs=2) as ep,
            tc.tile_pool(name="tokp", bufs=3) as tokp,
            tc.tile_pool(name="ohp", bufs=3) as ohp,
            tc.tile_pool(name="postp", bufs=2) as pop,
            tc.tile_pool(name="psbig", bufs=2, space="PSUM") as ps_big,
            tc.tile_pool(name="pssm", bufs=3, space="PSUM") as ps_sm,
            tc.tile_pool(name="psbin", bufs=2, space="PSUM") as ps_bin,
        ):
            # ---- DRAM scratch ----------------------------------------------
            ag0u_in = dram.tile([P, LUP], dt.float32, name="ag0u_in")
            ag0u = dram.tile([NC, P, LUP], dt.float32, addr_space="Shared", name="ag0u")
            ag0i_in = dram.tile([64, LIP], dt.float32, name="ag0i_in")
            ag0i = dram.tile([NC, 64, LIP], dt.float32, addr_space="Shared", name="ag0i")
            ag1u_in = dram.tile([P, LUP], dt.float32, name="ag1u_in")
            ag1u = dram.tile([NC, P, LUP], dt.float32, addr_space="Shared", name="ag1u")
            ag1i_in = dram.tile([P, LIP], dt.float32, name="ag1i_in")
            ag1i = dram.tile([NC, P, LIP], dt.float32, addr_space="Shared", name="ag1i")
            t_u = dram.tile([NUP, 512], dt.float16, name="t_u")
            t_i = dram.tile([NIP, 256], dt.float16, name="t_i")
            q_u = dram.tile([NUP, P], dt.float32, name="q_u")
            q_i = dram.tile([NIP, P], dt.float16, name="q_i")
            hloc = dram.tile([P, LUP + LIP], dt.float32, name="hloc")

            # ---- resident constants ----------------------------------------
            def load_const(name, src_ap, shape, dtype=dt.float32):
                raw = hp.tile(shape, dtype, tag="craw", name=f"{name}_r", bufs=2)
                nc.sync.dma_start(out=raw[:], in_=src_ap)
                sb = cp.tile(shape, dtype, name=name)
                nc.vector.tensor_copy(sb[:], raw[:])
                return sb

            def load_bcast(name, src_ap, width):
                raw = hp.tile([1, width], dt.float32, tag="craw1", name=f"{name}_r",
                              bufs=2)
                nc.sync.dma_start(out=raw[:], in_=src_ap)
                sb = cp.tile([P, width], dt.float32, name=name)
                nc.gpsimd.partition_broadcast(sb[:], raw[:])
                return sb

            winu_sb = load_const("winu", w_in_u[:, :], [P, P])
            wini_sb = load_const("wini", w_in_i[:, :], [64, P])
            binu_sb = load_const("binu", b_in_u[:, :], [P, 1])
            bini_sb = load_const("bini", b_in_i[:, :], [P, 1])
            wlin_sb = load_const("wlin", w_lin[:, :], [P, P])
            blin_sb = load_bcast("blin", b_lin[:, :], P)
            id_sb = load_const("ident", ident[:, :], [P, P])
            wbu_sb = [load_const(f"wbu{l}", wbig_u[l, :, :], [P, 640]) for l in range(L)]
            bbu_sb = [load_bcast(f"bbu{l}", bbig_u[l, :, :], 640) for l in range(L)]
            wbi_sb = [load_const(f"wbi{l}", wbig_i[l, :, :], [P, 384]) for l in range(L)]
            bbi_sb = [load_bcast(f"bbi{l}", bbig_i[l, :, :], 384) for l in range(L)]
            wou_sb = [load_const(f"wou{l}", w_out_u[l, :, :], [P, P]) for l in range(L)]
            woi_sb = [load_const(f"woi{l}", w_out_i[l, :, :], [P, P]) for l in range(L)]
            bou_sb = [load_const(f"bou{l}", b_out_u[l, :, :], [P, 1]) for l in range(L)]
            boi_sb = [load_const(f"boi{l}", b_out_i[l, :, :], [P, 1]) for l in range(L)]

            iot = cp.tile([P, P], dt.float32, name="iot")
            nc.gpsimd.iota(iot[:], pattern=[[1, P]], base=0, channel_multiplier=0,
                           allow_small_or_imprecise_dtypes=True)

            idx_sb = {}
            for st in (ui, iu, uu):
                d = {}
                for k in ("src", "qdst"):
                    w = st.n_tok // 16
                    t = cp.tile([P, w], dt.int16, name=f"{st.name}_{k}_sb")
                    for r in range(8):
                        nc.sync.dma_start(out=t[16 * r:16 * (r + 1), :],
                                          in_=idx_in[st.name][k][:, :])
                    d[k] = t
                t = cp.tile([P, st.n_tok // P], dt.float32,
                            name=f"{st.name}_dstv_sb")
                nc.sync.dma_start(out=t[:], in_=idx_in[st.name]["dstv"][:, :])
                d["dstv"] = t
                idx_sb[st.name] = d
            iukvpar_sb = cp.tile([P, iu.n_tok // P], dt.float32, name="iukvpar_sb")
            nc.sync.dma_start(out=iukvpar_sb[:], in_=idx_in["iu"]["kvpar"][:, :])
            uiqpar_sb = cp.tile([P, ui.n_tok // P], dt.float32, name="uiqpar_sb")
            nc.sync.dma_start(out=uiqpar_sb[:], in_=idx_in["ui"]["qpar"][:, :])
            idx_sb["iu"]["kvpar"] = iukvpar_sb
            idx_sb["ui"]["qpar"] = uiqpar_sb

            # ---- local h (layer-0 activations) into DRAM hloc ---------------
            for (xin, win_sb, bin_sb, pdim, ncols, coff) in (
                    (xu0, winu_sb, binu_sb, P, LUP, 0),
                    (xi0, wini_sb, bini_sb, 64, LIP, LUP)):
                for j in range(ncols // 512):
                    xr = hp.tile([P, 512], dt.float32, tag="xr", name="xr")
                    nc.sync.dma_start(out=xr[0:pdim, :],
                                      in_=xin[:, j * 512:(j + 1) * 512])
                    psh = ps_big.tile([P, 512], dt.float32, space="PSUM",
                                      tag="big", name="psh")
                    nc.tensor.matmul(out=psh[:], lhsT=win_sb[:],
                                     rhs=xr[0:pdim, :], start=True, stop=True)
                    ht = hp.tile([P, 512], dt.float32, tag="ht", name="ht")
                    nc.vector.tensor_scalar(ht[:], psh[:], bin_sb[:, 0:1], 0.0,
                                            OP.add, OP.max)
                    nc.sync.dma_start(
                        out=hloc[:, coff + j * 512:coff + (j + 1) * 512],
                        in_=ht[:])

            # ---- AllGather #0 ----------------------------------------------
            rg = [list(range(NC))]
            nc.sync.dma_start(out=ag0u_in[:], in_=xu0[:, :])
            nc.sync.dma_start(out=ag0i_in[:], in_=xi0[:, :])
            nc.gpsimd.collective_compute(
                "AllGather", OP.bypass, replica_groups=rg,
                ins=[ag0u_in[:].opt()], outs=[ag0u[:].opt()])
            nc.gpsimd.collective_compute(
                "AllGather", OP.bypass, replica_groups=rg,
                ins=[ag0i_in[:].opt()], outs=[ag0i[:].opt()])

            # =================================================================
            def stage1(l):
                # users
                for cc in range(NUP // 512):
                    if l == 0:
                        rank, sub = divmod(cc, LUP // 512)
                        xr = hp.tile([P, 512], dt.float32, tag="xr", name="xr_u")
                        nc.sync.dma_start(
                            out=xr[:],
                            in_=ag0u[rank, :, sub * 512:(sub + 1) * 512])
                        psh = ps_big.tile([P, 512], dt.float32, space="PSUM",
                                          tag="big", name="psh_u")
                        nc.tensor.matmul(out=psh[:], lhsT=winu_sb[:], rhs=xr[:],
                                         start=True, stop=True)
                        ht = hp.tile([P, 512], dt.float32, tag="ht", name="ht_u")
                        nc.vector.tensor_scalar(ht[:], psh[:], binu_sb[:, 0:1],
                                                0.0, OP.add, OP.max)
                    else:
                        rank, sub = divmod(cc, LUP // 512)
                        ht = hp.tile([P, 512], dt.float32, tag="ht", name="ht_u2")
                        nc.sync.dma_start(
                            out=ht[:],
                            in_=ag1u[rank, :, sub * 512:(sub + 1) * 512])
                    for j in range(4):
                        node0 = cc * 512 + j * P
                        pst = ps_big.tile([P, 512], dt.float32, space="PSUM",
                                          tag="big", name="pst_u")
                        nc.tensor.matmul(out=pst[:], lhsT=ht[:, j * P:(j + 1) * P],
                                         rhs=wbu_sb[l][:, 0:512],
                                         start=True, stop=True)
                        tt = s1p.tile([P, 512], dt.float16, tag="tt", name="tt_u")
                        nc.scalar.activation(tt[:], pst[:], AF.Copy)
                        nc.sync.dma_start(out=t_u[node0:node0 + P, :], in_=tt[:])
                        psq = ps_sm.tile([P, P], dt.float32, space="PSUM",
                                         tag="sm", name="psq_u")
                        nc.tensor.matmul(out=psq[:], lhsT=ht[:, j * P:(j + 1) * P],
                                         rhs=wbu_sb[l][:, 512:640],
                                         start=True, stop=True)
                        qt = s1p.tile([P, P], dt.float32, tag="qt", name="qt_u")
                        nc.scalar.activation(qt[:], psq[:], AF.Copy)
                        nc.sync.dma_start(out=q_u[node0:node0 + P, :], in_=qt[:])
                # items
                for cc in range(NIP // 512):
                    rank, sub = divmod(cc, LIP // 512)
                    if l == 0:
                        xr = hp.tile([P, 512], dt.float32, tag="xr", name="xr_i")
                        nc.sync.dma_start(
                            out=xr[0:64, :],
                            in_=ag0i[rank, :, sub * 512:(sub + 1) * 512])
                        psh = ps_big.tile([P, 512], dt.float32, space="PSUM",
                                          tag="big", name="psh_i")
                        nc.tensor.matmul(out=psh[:], lhsT=wini_sb[:],
                                         rhs=xr[0:64, :], start=True, stop=True)
                        ht = hp.tile([P, 512], dt.float32, tag="ht", name="ht_i")
                        nc.vector.tensor_scalar(ht[:], psh[:], bini_sb[:, 0:1],
                                                0.0, OP.add, OP.max)
                    else:
                        ht = hp.tile([P, 512], dt.float32, tag="ht", name="ht_i2")
                        nc.sync.dma_start(
                            out=ht[:],
                            in_=ag1i[rank, :, sub * 512:(sub + 1) * 512])
                    for j in range(4):
                        node0 = cc * 512 + j * P
                        pst = ps_big.tile([P, 384], dt.float32, space="PSUM",
                                          tag="big", name="pst_i")
                        nc.tensor.matmul(out=pst[:], lhsT=ht[:, j * P:(j + 1) * P],
                                         rhs=wbi_sb[l][:, :],
                                         start=True, stop=True)
                        tt = s1p.tile([P, 384], dt.float16, tag="tti", name="tt_i")
                        nc.scalar.activation(tt[:], pst[:], AF.Copy)
                        nc.sync.dma_start(out=t_i[node0:node0 + P, :],
                                          in_=tt[:, 0:256])
                        nc.sync.dma_start(out=q_i[node0:node0 + P, :],
                                          in_=tt[:, 256:384])

            # =================================================================
            t_i_pair = t_i[:].rearrange("(a b) c -> a (b c)", b=2)   # [NIP/2, 512]
            q_i_pair = q_i[:].rearrange("(a b) c -> a (b c)", b=2)   # [NIP/2, 256]

            def emit_chunk(st, ci, tok_tiles, l):
                """Gather + bias + score + exp + weighted-value for one chunk."""
                name = st.name
                sb = idx_sb[name]
                i16 = slice(ci * CH * 8, (ci + 1) * CH * 8)
                i128 = slice(ci * CH, (ci + 1) * CH)

                if name == "ui":
                    kvbias = bbu_sb[l][:, 0:256]
                    qbias = bbi_sb[l][:, 256:384]
                elif name == "iu":
                    kvbias = bbi_sb[l][:, 0:256]
                    qbias = bbu_sb[l][:, 512:640]
                else:
                    kvbias = bbu_sb[l][:, 256:512]
                    qbias = bbu_sb[l][:, 512:640]
                kvbias_b = kvbias.unsqueeze(1).broadcast_to([P, CH, 256])
                qbias_b = qbias.unsqueeze(1).broadcast_to([P, CH, 128])

                # --- kv gather (+ parity select for the paired item table) ---
                if name == "iu":
                    kv2 = ep.tile([P, CH, 512], dt.float16, tag="kv2", name="kv2")
                    nc.gpsimd.dma_gather(kv2[:], t_i_pair, sb["src"][:, i16],
                                         TC, TC, 512)
                    par_b = idx_sb["iu"]["kvpar"][:, i128].unsqueeze(2) \
                        .broadcast_to([P, CH, 256])
                    dif = ep.tile([P, CH, 256], dt.float32, tag="t256a", name="dif")
                    nc.vector.tensor_sub(dif[:], kv2[:, :, 256:512],
                                         kv2[:, :, 0:256])
                    nc.vector.tensor_mul(dif[:], dif[:], par_b)
                    kvf = ep.tile([P, CH, 256], dt.float32, tag="kvf", name="kvf")
                    nc.vector.tensor_add(kvf[:], dif[:], kv2[:, :, 0:256])
                    kv = kvf[:]
                else:
                    kvt = ep.tile([P, CH, 256], dt.float16, tag="kv2", name="kvt")
                    col0 = 0 if name == "ui" else 256
                    nc.gpsimd.dma_gather(kvt[:], t_u[:, col0:col0 + 256],
                                         sb["src"][:, i16], TC, TC, 256,
                                         elem_step=512)
                    kvf = ep.tile([P, CH, 256], dt.float32, tag="kvf", name="kvf2")
                    nc.vector.tensor_add(kvf[:], kvt[:], kvbias_b)
                    kv = kvf[:]
                if name == "iu":
                    nc.vector.tensor_add(kv, kv, kvbias_b)

                # --- q gather (+ parity select for the paired item q table) ---
                if name == "ui":
                    qg2 = ep.tile([P, CH, 256], dt.float16, tag="qg2h", name="qg2")
                    nc.gpsimd.dma_gather(qg2[:], q_i_pair, sb["qdst"][:, i16],
                                         TC, TC, 256)
                    qpar_b = idx_sb["ui"]["qpar"][:, i128].unsqueeze(2) \
                        .broadcast_to([P, CH, 128])
                    qdif = ep.tile([P, CH, 128], dt.float32, tag="t128a", name="qdif")
                    nc.vector.tensor_sub(qdif[:], qg2[:, :, 128:256],
                                         qg2[:, :, 0:128])
                    nc.vector.tensor_mul(qdif[:], qdif[:], qpar_b)
                    qgf = ep.tile([P, CH, 128], dt.float32, tag="qgf", name="qgf")
                    nc.vector.tensor_add(qgf[:], qdif[:], qg2[:, :, 0:128])
                    qg = qgf[:]
                else:
                    qgt = ep.tile([P, CH, 128], dt.float32, tag="qg2h", name="qgt")
                    nc.gpsimd.dma_gather(qgt[:], q_u[:, :], sb["qdst"][:, i16],
                                         TC, TC, 128)
                    qgf = ep.tile([P, CH, 128], dt.float32, tag="qgf", name="qgf2")
                    nc.vector.tensor_add(qgf[:], qgt[:], qbias_b)
                    qg = qgf[:]
                if name == "ui":
                    nc.vector.tensor_add(qg, qg, qbias_b)

                # --- scores s = sum_d q*k per head; e = exp(s) into tok ---
                prod = ep.tile([P, CH, 128], dt.float32, tag="t128b", name="prod")
                nc.vector.tensor_mul(prod[:], qg, kv[:, :, 0:128])
                s = ep.tile([P, CH, 8], dt.float32, tag="s", name="s")
                pr4 = prod[:].rearrange("p n (h d) -> p n h d", h=8)
                nc.vector.tensor_reduce(s[:], pr4, mybir.AxisListType.X, OP.add)
                tok = tokp.tile([P, CH, 136], dt.float32, tag="tok", name="tok")
                nc.scalar.activation(tok[:, :, 128:136], s[:], AF.Exp)
                tok4 = tok[:, :, 0:128].rearrange("p n (h d) -> p n h d", h=8)
                kvv4 = kv[:, :, 128:256].rearrange("p n (h d) -> p n h d", h=8)
                e4 = tok[:, :, 128:136].unsqueeze(3).broadcast_to([P, CH, 8, 16])
                nc.vector.tensor_mul(tok4, kvv4, e4)

                # --- one-hot block for the whole chunk ---
                oh = ohp.tile([P, CH, P], dt.float32, tag="oh", name="oh")
                iot_b = iot[:, :].unsqueeze(1).broadcast_to([P, CH, P])
                dv_b = sb["dstv"][:, i128].unsqueeze(2).broadcast_to([P, CH, P])
                nc.vector.tensor_tensor(oh[:], iot_b, dv_b, OP.is_equal)
                tok_tiles[ci] = (tok, oh)

            def post_bin(l, kind, b, psum_bin):
                """Divide, gelu, W_out, gated skip; route result."""
                g_val = g_gate[(kind, l)]
                wout = (wou_sb if kind == "u" else woi_sb)[l]
                bout = (bou_sb if kind == "u" else boi_sb)[l]
                hoff = (0 if kind == "u" else LUP) + b * P
                fl = pop.tile([P, 136], dt.float32, tag="fl2", name="fl2")
                nc.scalar.activation(fl[:], psum_bin[:], AF.Copy)
                den = pop.tile([P, 8], dt.float32, tag="den", name="den")
                nc.vector.tensor_scalar_add(den[:], fl[:, 128:136], 1e-16)
                rec = pop.tile([P, 8], dt.float32, tag="rec", name="rec")
                nc.vector.reciprocal(rec[:], den[:])
                gsrc = pop.tile([P, 8, 18], dt.float32, tag="gsrc", name="gsrc")
                fl4 = fl[:, 0:128].rearrange("p (h d) -> p h d", h=8)
                rec4 = rec[:].unsqueeze(2).broadcast_to([P, 8, 16])
                nc.vector.tensor_mul(gsrc[:, :, 0:16], fl4, rec4)
                G = pop.tile([P, P], dt.float32, tag="G", name="G")
                nc.scalar.activation(G[:], gsrc[:, :, 0:16], AF.Gelu)
                gtp = ps_sm.tile([P, P], dt.float32, space="PSUM", tag="sm",
                                 name="gtp")
                nc.tensor.transpose(gtp[:], G[:], id_sb[:])
                gt = pop.tile([P, P], dt.float32, tag="gt", name="gt")
                nc.vector.tensor_copy(gt[:], gtp[:])
                aps = ps_sm.tile([P, P], dt.float32, space="PSUM", tag="sm",
                                 name="aps")
                nc.tensor.matmul(out=aps[:], lhsT=wout[:], rhs=gt[:],
                                 start=True, stop=True)
                t1 = pop.tile([P, P], dt.float32, tag="t1", name="t1")
                nc.vector.tensor_scalar(t1[:], aps[:], bout[:, 0:1], g_val,
                                        OP.add, OP.mult)
                xo = pop.tile([P, P], dt.float32, tag="xo", name="xo")
                if l == 0:
                    nc.sync.dma_start(out=xo[:], in_=hloc[:, hoff:hoff + P])
                else:
                    srcb = ag1u_in if kind == "u" else ag1i_in
                    off = b * P
                    nc.sync.dma_start(out=xo[:], in_=srcb[:, off:off + P])
                x2 = pop.tile([P, P], dt.float32, tag="x2", name="x2")
                nc.vector.scalar_tensor_tensor(
                    out=x2[:], in0=xo[:], scalar=1.0 - g_val, in1=t1[:],
                    op0=OP.mult, op1=OP.add)
                x2r = pop.tile([P, P], dt.float32, tag="x2r", name="x2r")
                nc.scalar.activation(x2r[:], x2[:], AF.Relu)
                if l == 0:
                    dst = ag1u_in if kind == "u" else ag1i_in
                    nc.sync.dma_start(out=dst[:, b * P:(b + 1) * P], in_=x2r[:])
                else:
                    fps = ps_sm.tile([P, P], dt.float32, space="PSUM", tag="sm",
                                     name="fps")
                    nc.tensor.matmul(out=fps[:], lhsT=x2r[:], rhs=wlin_sb[:],
                                     start=True, stop=True)
                    fo = pop.tile([P, P], dt.float32, tag="fo", name="fo")
                    nc.vector.tensor_add(fo[:], fps[:], blin_sb[:])
                    row0 = (0 if kind == "u" else LUP) + b * P
                    nc.sync.dma_start(out=out[row0:row0 + P, 0:64],
                                      in_=fo[:, 0:64])

            def edge_phase(l):
                sinks = [("i", NBI, [ui]), ("u", NBU, [iu, uu])]
                for kind, n_bins, sts in sinks:
                    tok_tiles = {st.name: {} for st in sts}
                    for b in range(n_bins):
                        mlist = [(st, g) for st in sts for g in st.bin_slices[b]]
                        pb = ps_bin.tile([P, 136], dt.float32, space="PSUM",
                                         tag="pb", name=f"pb_{kind}")
                        for k, (st, g) in enumerate(mlist):
                            ci = g // CH
                            if ci not in tok_tiles[st.name]:
                                emit_chunk(st, ci, tok_tiles[st.name], l)
                            tok, oh = tok_tiles[st.name][ci]
                            nc.tensor.matmul(
                                out=pb[:], lhsT=oh[:, g % CH, :],
                                rhs=tok[:, g % CH, :],
                                start=(k == 0), stop=(k == len(mlist) - 1))
                        post_bin(l, kind, b, pb)

            # =================================================================
            for l in range(L):
                stage1(l)
                edge_phase(l)
                if l == 0:
                    nc.gpsimd.collective_compute(
                        "AllGather", OP.bypass, replica_groups=rg,
                        ins=[ag1u_in[:].opt()], outs=[ag1u[:].opt()])
                    nc.gpsimd.collective_compute(
                        "AllGather", OP.bypass, replica_groups=rg,
                        ins=[ag1i_in[:].opt()], outs=[ag1i[:].opt()])

    nc.compile()
    return nc


# ---------------------------------------------------------------------------
# host orchestration
# ---------------------------------------------------------------------------

def _prepare(inp):
    """Fold weights, build streams/plans and per-core in_maps."""
    f32 = np.float32
    inv_sqrt_d = f32(1.0 / np.sqrt(f32(D)))
    A_k = np.asarray(inp["A_k"], f32)
    A_v = np.asarray(inp["A_v"], f32)
    p_rel = np.asarray(inp["p_rel"], f32)

    wbig_u = np.zeros((L, P, 640), f32)
    bbig_u = np.zeros((L, 1, 640), f32)
    wbig_i = np.zeros((L, P, 384), f32)
    bbig_i = np.zeros((L, 1, 384), f32)
    for l in range(L):
        Wk_u, Wq_u, Wv_u = np.split(np.asarray(inp["W_kqv_user"][l], f32), 3, axis=1)
        bk_u, bq_u, bv_u = np.split(np.asarray(inp["b_kqv_user"][l], f32), 3)
        Wk_i, Wq_i, Wv_i = np.split(np.asarray(inp["W_kqv_item"][l], f32), 3, axis=1)
        bk_i, bq_i, bv_i = np.split(np.asarray(inp["b_kqv_item"][l], f32), 3)

        def bk(r):
            return _blockdiag(A_k[l, r] * (p_rel[l, r] * inv_sqrt_d)[:, None, None])

        Bk0, Bk1, Bk2 = bk(0), bk(1), bk(2)
        Bv0, Bv1, Bv2 = (_blockdiag(A_v[l, r]) for r in range(3))
        wbig_u[l] = np.concatenate(
            [Wk_u @ Bk0, Wv_u @ Bv0, Wk_u @ Bk2, Wv_u @ Bv2, Wq_u], axis=1)
        bb = np.concatenate([bk_u @ Bk0, bv_u @ Bv0, bk_u @ Bk2, bv_u @ Bv2, bq_u])
        bbig_u[l] = bb
        wbig_i[l] = np.concatenate([Wk_i @ Bk1, Wv_i @ Bv1, Wq_i], axis=1)
        bb = np.concatenate([bk_i @ Bk1, bv_i @ Bv1, bq_i])
        bbig_i[l] = bb

    g_gate = {}
    for l in range(L):
        g_gate[("u", l)] = float(1.0 / (1.0 + np.exp(-f32(inp["skip_user"][l]))))
        g_gate[("i", l)] = float(1.0 / (1.0 + np.exp(-f32(inp["skip_item"][l]))))

    # ---- edge streams ------------------------------------------------------
    src_ui = np.asarray(inp["edge_src_ui"], np.int64)
    dst_ui = np.asarray(inp["edge_dst_ui"], np.int64)
    src_iu = np.asarray(inp["edge_src_iu"], np.int64)
    dst_iu = np.asarray(inp["edge_dst_iu"], np.int64)
    src_uu = np.asarray(inp["edge_src_uu"], np.int64)
    dst_uu = np.asarray(inp["edge_dst_uu"], np.int64)

    ui = Stream("ui", NBI)
    iu = Stream("iu", NBU)
    uu = Stream("uu", NBU)

    core_of_item = dst_ui // LI
    core_of_user_iu = dst_iu // LU
    core_of_user_uu = dst_uu // LU

    def counts(core_sel, dst_loc, n_bins):
        res = []
        for c in range(NC):
            m = core_sel == c
            res.append(np.bincount(dst_loc[m] // P, minlength=n_bins))
        return res

    ui.plan(counts(core_of_item, dst_ui % LI, NBI))
    iu.plan(counts(core_of_user_iu, dst_iu % LU, NBU))
    uu.plan(counts(core_of_user_uu, dst_uu % LU, NBU))

    for c in range(NC):
        m = core_of_item == c
        sp = _pad_users(src_ui[m])
        qp = _pad_items(dst_ui[m])
        ui.fill(sp, None, qp >> 1, (qp & 1).astype(np.float32), dst_ui[m] % LI)
        m = core_of_user_iu == c
        sp = _pad_items(src_iu[m])
        iu.fill(sp >> 1, (sp & 1).astype(np.float32),
                _pad_users(dst_iu[m]), None, dst_iu[m] % LU)
        m = core_of_user_uu == c
        uu.fill(_pad_users(src_uu[m]), None, _pad_users(dst_uu[m]), None,
                dst_uu[m] % LU)

    # ---- per-core in_maps --------------------------------------------------
    x_user = np.asarray(inp["x_user"], f32)
    x_item = np.asarray(inp["x_item"], f32)
    in_maps = []
    shared = {
        "w_in_u": np.ascontiguousarray(np.asarray(inp["W_in_user"], f32)),
        "w_in_i": np.ascontiguousarray(np.asarray(inp["W_in_item"], f32)),
        "b_in_u": np.asarray(inp["b_in_user"], f32).reshape(P, 1),
        "b_in_i": np.asarray(inp["b_in_item"], f32).reshape(P, 1),
        "wbig_u": wbig_u, "bbig_u": bbig_u,
        "wbig_i": wbig_i, "bbig_i": bbig_i,
        "w_out_u": np.asarray(inp["W_out_user"], f32),
        "w_out_i": np.asarray(inp["W_out_item"], f32),
        "b_out_u": np.asarray(inp["b_out_user"], f32).reshape(L, P, 1),
        "b_out_i": np.asarray(inp["b_out_item"], f32).reshape(L, P, 1),
        "w_lin": np.concatenate(
            [np.asarray(inp["W_lin"], f32),
             np.zeros((P, 64), f32)], axis=1),
        "b_lin": np.concatenate([np.asarray(inp["b_lin"], f32),
                                 np.zeros(64, f32)]).reshape(1, P),
        "ident": np.eye(P, dtype=f32),
    }
    for c in range(NC):
        xu = np.zeros((P, LUP), f32)
        xu[:, :LU] = x_user[c * LU:(c + 1) * LU].T
        xi = np.zeros((64, LIP), f32)
        xi[:, :LI] = x_item[c * LI:(c + 1) * LI].T
        m = dict(shared)
        m["xu0"] = xu
        m["xi0"] = xi
        for st in (ui, iu, uu):
            m[f"{st.name}_src"] = st.src16[c]
            m[f"{st.name}_qdst"] = st.qdst16[c]
            m[f"{st.name}_dstv"] = st.dstv[c]
        m["iu_kvpar"] = iu.kvpar[c]
        m["ui_qpar"] = ui.qpar[c]
        in_maps.append(m)

    return {"ui": ui, "iu": iu, "uu": uu}, g_gate, in_maps


def kernel(**inp):
    global _HW_NS_TOTAL, _LAST_RES
    import time
    from concourse import bass_utils

    streams, g_gate, in_maps = _prepare(inp)

    key = (streams["ui"].n_tok, streams["iu"].n_tok, streams["uu"].n_tok,
           tuple(g_gate.values()))
    if key not in _prog_cache:
        _prog_cache.clear()
        _prog_cache[key] = _build_program(streams, g_gate)
    nc = _prog_cache[key]

    t0 = time.time()
    res = bass_utils.run_bass_kernel_spmd(nc, in_maps, core_ids=list(range(NC)))
    dt_ns = int((time.time() - t0) * 1e9)
    if res.exec_time_ns:
        dt_ns = int(res.exec_time_ns)
    _HW_NS_TOTAL += dt_ns
    _LAST_RES = res

    out = np.zeros((NU + NI, 64), np.float32)
    for c in range(NC):
        o = res.results[c]["out"]
        out[c * LU:(c + 1) * LU] = o[0:LU, :]
        out[NU + c * LI:NU + (c + 1) * LI] = o[LUP:LUP + LI, :]
    return out
